# revision 1
# baseline (speedup 1.0000x reference)
"""Self-contained Trainium2 kernel for nn_Attention_58033598104213.

GQA causal attention block (B=2, T=2048, d_model=2048, 16 Q heads / 4 KV
heads, head_dim=128, RoPE, causal SDPA, output projection).

Sharding: 8 NeuronCores = 2 batches x 4 head-groups. Core (b, g) computes
all T queries of batch b for Q-heads 4g..4g+3 (which share KV head g) and
the partial product against wo's matching row slice; the host sums the 4
partials per batch (row-parallel wo => partial-sum gather). No collectives.

On-device pipeline per core (bf16 compute, fp32 PSUM accumulation):
  1. q/k/v projections from pre-transposed x, RoPE fused into PSUM-evict
  2. flash-style causal attention in score-transposed layout [keys, queries];
     exp on ScalarE, diagonal blocks trimmed to their causally-live width,
     softmax denominators as a bf16 pair-tree on VectorE + one f32r
     ones-matmul partition-reduce, 1/x as exp(-ln x) on ScalarE,
     partition-broadcast via rank-1 matmul
  3. output projection with back-to-back PSUM accumulation chains, fp32 out
"""
import numpy as np
import ml_dtypes
import orjson
import concourse.bass as bass
import concourse.mybir as mybir
import concourse.tile as tile
from concourse.masks import make_identity
from concourse.bass_utils import run_bass_kernel_spmd

# ---------------------------------------------------------------------------
# Walrus in this image accepts only one sem-wait per instruction; the Tile
# framework's final drain carries several. Split excess waits onto preceding
# NoOps on the same engine (in-order execution preserves the AND semantics).
import orjson

_MARK = "_bir_wait_split_patched"


def split_waits(bir: bytes, maxw: int = 1) -> bytes:
    m = orjson.loads(bir)
    n_split = 0

    def fix_instructions(insts: list) -> list:
        nonlocal n_split
        out = []
        for ins in insts:
            si = ins.get("sync_info")
            waits = (si or {}).get("on_wait") or []
            if len(waits) > maxw:
                n_split += 1
                head, rest = waits[: len(waits) - maxw], waits[len(waits) - maxw :]
                for k in range(0, len(head), maxw):
                    out.append(
                        {
                            "debug": ins.get("debug", 0),
                            "engine": ins["engine"],
                            "ins": [],
                            "name": f"{ins['name']}-wsplit{k}",
                            "opcode": "NoOp",
                            "outs": [],
                            "sync_info": {
                                "on_update": [],
                                "on_wait": head[k : k + maxw],
                            },
                        }
                    )
                si["on_wait"] = rest
            out.append(ins)
        return out

    def walk(o):
        if isinstance(o, dict):
            if isinstance(o.get("instructions"), list):
                o["instructions"] = fix_instructions(o["instructions"])
            for v in o.values():
                walk(v)
        elif isinstance(o, list):
            for v in o:
                walk(v)

    walk(m)
    return orjson.dumps(m)


def patch_nc(nc, maxw: int = 1):
    if getattr(nc, _MARK, False):
        return nc
    orig = nc.to_json_bytes

    def wrapped(*a, **kw):
        return split_waits(orig(*a, **kw), maxw=maxw)

    nc.to_json_bytes = wrapped
    setattr(nc, _MARK, True)
    return nc


# ---------------------------------------------------------------------------
import numpy as np
import ml_dtypes
import concourse.bass as bass
import concourse.mybir as mybir
import concourse.tile as tile
from concourse.masks import make_identity

F32 = mybir.dt.float32
BF16 = mybir.dt.bfloat16
AF = mybir.ActivationFunctionType

P = 128
T = 2048
D = 2048
NT = D // P
HQ = 4
HD = 128
MB = 512
NMB = T // MB
QC = 512
NQC = T // QC
KB = 128
SCALE = float(1.0 / np.sqrt(HD))
HALF = HD // 2


def _rope2(nc, pool, dst, sl, p, cos2, sin2, width):
    """dst[:, sl] = bf16(p*cos2 + swap(p)*sin2); p is fp32 PSUM."""
    sw = pool.tile([P, width], F32, tag="rope_sw", name="sw")
    nc.vector.tensor_copy(sw[0:HALF, :], p[HALF:P, :])
    nc.vector.tensor_copy(sw[HALF:P, :], p[0:HALF, :])
    t2 = pool.tile([P, width], F32, tag="rope_t2", name="t2")
    nc.vector.tensor_mul(t2[:], sw[:], sin2[:, sl])
    t3 = pool.tile([P, width], F32, tag="rope_t3", name="t3")
    nc.vector.tensor_mul(t3[:], p[:], cos2[:, sl])
    nc.vector.tensor_add(dst[:, sl], t3[:], t2[:])


def _phase1(nc, tc, tensors, qT_s, kT_s, v_s, mask_s, ones_c, ones_r, ident):
    (xT_t, wq, wk_t, wv_t, maskp, ones_col_d, ones_row_d, cos2d, sin2d) = tensors
    with (
        tc.tile_pool(name="ph1_w", bufs=1) as w1,
        tc.tile_pool(name="ph1_xt", bufs=3) as xtp,
        tc.tile_pool(name="ph1_ps", bufs=1, space="PSUM") as ps1,
        tc.tile_pool(name="ph1_ptr", bufs=2, space="PSUM") as psp,
        tc.tile_pool(name="ph1_tmp", bufs=3) as rtp,
    ):
        xt0 = xtp.tile([P, NT, MB], BF16, tag="xt", name="xt0")
        nc.sync.dma_start(out=xt0[:, 0 : NT // 2, :], in_=xT_t[:, 0 : NT // 2, 0:MB])
        nc.sync.dma_start(out=xt0[:, NT // 2 :, :], in_=xT_t[:, NT // 2 :, 0:MB])
        wk_s = w1.tile([P, NT, HD], BF16, name="wk_s")
        nc.sync.dma_start(out=wk_s[:], in_=wk_t)
        wv_s = w1.tile([P, NT, HD], BF16, name="wv_s")
        nc.sync.dma_start(out=wv_s[:], in_=wv_t)
        cos_s = w1.tile([P, T], F32, name="cos_s")
        nc.sync.dma_start(out=cos_s[:], in_=cos2d[:])
        sin_s = w1.tile([P, T], F32, name="sin_s")
        nc.sync.dma_start(out=sin_s[:], in_=sin2d[:])
        wq_s = w1.tile([P, NT, HQ * HD], BF16, name="wq_s")
        wq_t = wq.rearrange("(t p) n -> p t n", p=P)
        for h in range(HQ):
            nc.sync.dma_start(
                out=wq_s[:, :, h * HD : (h + 1) * HD],
                in_=wq_t[:, :, h * HD : (h + 1) * HD],
            )
        nc.sync.dma_start(out=mask_s[:], in_=maskp[:])
        nc.sync.dma_start(out=ones_c[:].bitcast(mybir.dt.float32r), in_=ones_col_d[:].bitcast(mybir.dt.float32r))
        nc.sync.dma_start(out=ones_r[:], in_=ones_row_d[:])
        make_identity(nc, ident[:])

        for m in range(NMB):
            sl = slice(m * MB, (m + 1) * MB)
            if m == 0:
                xt = xt0
            else:
                xt = xtp.tile([P, NT, MB], BF16, tag="xt", name=f"xt{m}")
                nc.sync.dma_start(out=xt[:, 0 : NT // 2, :], in_=xT_t[:, 0 : NT // 2, sl])
                nc.sync.dma_start(out=xt[:, NT // 2 :, :], in_=xT_t[:, NT // 2 :, sl])
            pk = ps1.tile([P, MB], F32, tag="pk", name=f"pk{m}")
            for t in range(NT):
                nc.tensor.matmul(
                    pk[:], wk_s[:, t, :], xt[:, t, :],
                    start=(t == 0), stop=(t == NT - 1),
                )
            _rope2(nc, rtp, kT_s, sl, pk, cos_s, sin_s, MB)
            pv = ps1.tile([P, MB], F32, tag="pv", name=f"pv{m}")
            for t in range(NT):
                nc.tensor.matmul(
                    pv[:], wv_s[:, t, :], xt[:, t, :],
                    start=(t == 0), stop=(t == NT - 1),
                )
            vt_stage = rtp.tile([P, MB], BF16, tag="vts", name=f"vts{m}")
            nc.scalar.copy(vt_stage[:], pv[:])
            for sub in range(MB // P):
                ptr = psp.tile([P, P], BF16, tag="ptr", name=f"ptr{m}_{sub}")
                nc.tensor.transpose(
                    ptr[:], vt_stage[:, sub * P : (sub + 1) * P], ident[:]
                )
                nc.vector.tensor_copy(v_s[:, m * (MB // P) + sub, :], ptr[:])
            for h in range(HQ):
                pq = ps1.tile([P, MB], F32, tag=f"pq{h}", name=f"pq{m}_{h}")
                for t in range(NT):
                    nc.tensor.matmul(
                        pq[:],
                        wq_s[:, t, h * HD : (h + 1) * HD],
                        xt[:, t, :],
                        start=(t == 0),
                        stop=(t == NT - 1),
                    )
                _rope2(nc, rtp, qT_s[h], sl, pq, cos_s, sin_s, MB)


def _attention(nc, tc, qT_s, kT_s, v_s, mask_s, ones_c, ones_r, ctx_s):
    F32R = mybir.dt.float32r

    def r(ap):
        return ap.bitcast(F32R)

    with (
        tc.tile_pool(name="ph2_work", bufs=6) as wk2,
        tc.tile_pool(name="ph2_cp", bufs=2, space="PSUM") as psc,
        tc.tile_pool(name="ph2_sp", bufs=3, space="PSUM") as pss,
        tc.tile_pool(name="ph2_sm", bufs=1, space="PSUM") as psm,
    ):
        for h in range(HQ):
            for c in range(NQC):
                nblk = 4 * (c + 1)
                cp = psc.tile([P, QC], F32, tag="cp", name=f"cp{h}_{c}")
                sumacc = wk2.tile([P, QC], F32, tag="sumacc", name=f"sa{h}_{c}")
                prev = None
                for j in range(nblk):
                    d = j - 4 * c  # >= 0 on diagonal blocks
                    q0 = 128 * d if d >= 0 else 0
                    n = QC - q0
                    qsl = slice(c * QC + q0, (c + 1) * QC)
                    sp = pss.tile([P, QC], F32, tag="sp", name=f"sp{h}_{c}_{j}")
                    nc.tensor.matmul(
                        sp[:, q0:QC],
                        kT_s[:, j * KB : (j + 1) * KB],
                        qT_s[h][:, qsl],
                        start=True, stop=True,
                    )
                    pT = wk2.tile([P, QC], BF16, tag="pT", name=f"pT{h}_{c}_{j}")
                    nc.scalar.activation(pT[:, q0:QC], sp[:, q0:QC], AF.Exp, scale=SCALE)
                    if d >= 0:
                        nc.vector.tensor_mul(
                            pT[:, q0:QC], pT[:, q0:QC], mask_s[:, 3 * KB : 3 * KB + n]
                        )
                    nc.tensor.matmul(
                        cp[:, q0:QC], v_s[:, j, :], pT[:, q0:QC],
                        start=(j == 0), stop=(j == nblk - 1),
                    )
                    # softmax denominators on DVE: pair off-diagonal blocks in
                    # bf16 (2x mode), accumulate pairs + diagonals in fp32
                    if d < 0 and prev is None:
                        prev = pT
                    elif d < 0:
                        pair = wk2.tile([P, QC], BF16, tag="pair", name=f"pp{h}_{c}_{j}")
                        nc.vector.tensor_add(pair[:], prev[:], pT[:])
                        prev = None
                        if j == 1:
                            nc.vector.tensor_copy(r(sumacc[:]), pair[:])
                        else:
                            nc.vector.tensor_add(r(sumacc[:]), sumacc[:], pair[:])
                    else:  # diagonal: accumulate live slice directly in fp32
                        if j == 0:
                            nc.vector.tensor_copy(r(sumacc[:]), pT[:])
                        else:
                            nc.vector.tensor_add(
                                r(sumacc[:, q0:QC]), sumacc[:, q0:QC], pT[:, q0:QC]
                            )
                # partition-reduce on PE (f32r), then 1/x = exp(-ln x) on ACT
                sm = psm.tile([1, QC], F32, tag="sm", name=f"sm{h}_{c}")
                nc.tensor.matmul(sm[:], r(ones_c[:]), r(sumacc[:]), start=True, stop=True)
                lns = wk2.tile([1, QC], F32, tag="lns", name=f"ln{h}_{c}")
                nc.scalar.activation(lns[:], sm[:], AF.Ln)
                rrow = wk2.tile([1, QC], BF16, tag="rrow", name=f"rr{h}_{c}")
                nc.scalar.activation(rrow[:], lns[:], AF.Exp, scale=-1.0)
                prb = psm.tile([P, QC], F32, tag="prb", name=f"prb{h}_{c}")
                nc.tensor.matmul(prb[:], ones_r[:], rrow[:], start=True, stop=True)
                rbc = wk2.tile([P, QC], F32, tag="rbc", name=f"rbc{h}_{c}")
                nc.scalar.copy(rbc[:], prb[:])
                nc.vector.tensor_mul(
                    ctx_s[h][:, c * QC : (c + 1) * QC], cp[:], rbc[:]
                )


def _oproj(nc, tc, ctx_s, wo_s, out):
    with (
        tc.tile_pool(name="ph3_ps", bufs=1, space="PSUM") as ps3,
        tc.tile_pool(name="ph3_out", bufs=6) as outp,
    ):
        for u in range(T // P):
            usl = slice(u * P, (u + 1) * P)
            po = [
                ps3.tile([P, 512], F32, tag=f"po{n}", name=f"po{u}_{n}")
                for n in range(4)
            ]
            for n in range(4):
                for h in range(HQ):
                    nc.tensor.matmul(
                        po[n][:],
                        ctx_s[h][:, usl],
                        wo_s[:, h, n * 512 : (n + 1) * 512],
                        start=(h == 0),
                        stop=(h == HQ - 1),
                    )
            for n in range(4):
                so = outp.tile([P, 512], F32, tag="so", name=f"so{u}_{n}")
                if n % 2 == 0:
                    nc.vector.tensor_copy(so[:], po[n][:])
                else:
                    nc.scalar.copy(so[:], po[n][:])
                nc.sync.dma_start(out=out[usl, n * 512 : (n + 1) * 512], in_=so[:])


def build():
    nc = bass.Bass()
    xT = nc.declare_dram_parameter("xT", [D, T], BF16, isOutput=False)
    wq = nc.declare_dram_parameter("wq", [D, HQ * HD], BF16, isOutput=False)
    wk = nc.declare_dram_parameter("wk", [D, HD], BF16, isOutput=False)
    wv = nc.declare_dram_parameter("wv", [D, HD], BF16, isOutput=False)
    wo = nc.declare_dram_parameter("wo", [HQ * HD, D], BF16, isOutput=False)
    cos2d = nc.declare_dram_parameter("cos2", [P, T], F32, isOutput=False)
    sin2d = nc.declare_dram_parameter("sin2", [P, T], F32, isOutput=False)
    maskp = nc.declare_dram_parameter("maskp", [P, 3 * KB + QC], BF16, isOutput=False)
    ones_col_d = nc.declare_dram_parameter("ones_col", [P, 1], F32, isOutput=False)
    ones_row_d = nc.declare_dram_parameter("ones_row", [1, P], BF16, isOutput=False)
    out = nc.declare_dram_parameter("out", [T, D], F32, isOutput=True)

    xT_t = xT.rearrange("(t p) k -> p t k", p=P)
    wk_t = wk.rearrange("(t p) n -> p t n", p=P)
    wv_t = wv.rearrange("(t p) n -> p t n", p=P)
    wo_t = wo.rearrange("(h p) n -> p h n", p=P)

    with tile.TileContext(nc) as tc, nc.allow_low_precision(reason="bf16 compute"):
        with tc.tile_pool(name="resident", bufs=1) as big:
            qT_s = [big.tile([P, T], BF16, tag=f"qT{h}", name=f"qT{h}") for h in range(HQ)]
            kT_s = big.tile([P, T], BF16, tag="kT", name="kT")
            v_s = big.tile([P, T // P, HD], BF16, tag="v", name="v")
            mask_s = big.tile([P, 3 * KB + QC], BF16, tag="mask", name="mask")
            ones_c = big.tile([P, 1], F32, tag="ones_c", name="ones_c")
            ones_r = big.tile([1, P], BF16, tag="ones_r", name="ones_r")
            ident = big.tile([P, P], BF16, tag="ident", name="ident")

            tensors = (xT_t, wq, wk_t, wv_t, maskp, ones_col_d, ones_row_d, cos2d, sin2d)
            _phase1(nc, tc, tensors, qT_s, kT_s, v_s, mask_s, ones_c, ones_r, ident)

            with tc.tile_pool(name="ph2_sb", bufs=1) as sb2:
                ctx_s = [
                    sb2.tile([P, T], BF16, tag=f"ctx{h}", name=f"ctx{h}")
                    for h in range(HQ)
                ]
                wo_s = sb2.tile([P, HQ, D], BF16, tag="wo", name="wo_s")
                nc.sync.dma_start(out=wo_s[:], in_=wo_t)

                _attention(nc, tc, qT_s, kT_s, v_s, mask_s, ones_c, ones_r, ctx_s)
                _oproj(nc, tc, ctx_s, wo_s, out)

    patch_nc(nc, maxw=1)
    return nc


def rope_tables():
    inv_freq = 1.0 / (10000.0 ** (np.arange(0, HD, 2, dtype=np.float64) / HD))
    t = np.arange(T, dtype=np.float64)
    freqs = np.outer(t, inv_freq)
    c = np.cos(freqs).T.astype(np.float32)
    s = np.sin(freqs).T.astype(np.float32)
    cos2 = np.concatenate([c, c], 0)
    sin2 = np.concatenate([-s, s], 0)
    return np.ascontiguousarray(cos2), np.ascontiguousarray(sin2)


def mask_pad():
    k = np.arange(P)[:, None]
    p = np.arange(3 * KB + QC)[None, :]
    return (p >= k + 3 * KB).astype(ml_dtypes.bfloat16)


def make_in_maps(x, wq, wk, wv, wo):
    bf = ml_dtypes.bfloat16
    cos2, sin2 = rope_tables()
    maskp = mask_pad()
    ones_col = np.ones((P, 1), np.float32)
    ones_row = np.ones((1, P), bf)
    xTs = [np.ascontiguousarray(x[b].T.astype(bf)) for b in range(2)]
    wqb, wkb, wvb, wob = (a.astype(bf) for a in (wq, wk, wv, wo))
    in_maps = []
    for c in range(8):
        b, g = divmod(c, 4)
        in_maps.append(
            {
                "xT": xTs[b],
                "wq": np.ascontiguousarray(wqb[:, 512 * g : 512 * (g + 1)]),
                "wk": np.ascontiguousarray(wkb[:, 128 * g : 128 * (g + 1)]),
                "wv": np.ascontiguousarray(wvb[:, 128 * g : 128 * (g + 1)]),
                "wo": np.ascontiguousarray(wob[512 * g : 512 * (g + 1), :]),
                "cos2": cos2,
                "sin2": sin2,
                "maskp": maskp,
                "ones_col": ones_col,
                "ones_row": ones_row,
            }
        )
    return in_maps


def combine_outputs(results):
    out = np.zeros((2, T, D), np.float32)
    for c in range(8):
        out[c // 4] += results[c]["out"]
    return out


_NC_CACHE = []


def kernel(x, wq, wk, wv, wo):
    x = np.asarray(x, dtype=np.float32)
    wq = np.asarray(wq, dtype=np.float32)
    wk = np.asarray(wk, dtype=np.float32)
    wv = np.asarray(wv, dtype=np.float32)
    wo = np.asarray(wo, dtype=np.float32)
    if not _NC_CACHE:
        _NC_CACHE.append(build())
    nc = _NC_CACHE[0]
    in_maps = make_in_maps(x, wq, wk, wv, wo)
    res = run_bass_kernel_spmd(nc, in_maps, core_ids=list(range(8)))
    return combine_outputs(res.results)



# revision 2
# speedup vs baseline: 1.0042x; 1.0042x over previous
"""Self-contained Trainium2 kernel for nn_Attention_58033598104213.

GQA causal attention block (B=2, T=2048, d_model=2048, 16 Q heads / 4 KV
heads, head_dim=128, RoPE, causal SDPA, output projection).

Sharding: 8 NeuronCores = 2 batches x 4 head-groups. Core (b, g) computes
all T queries of batch b for Q-heads 4g..4g+3 (which share KV head g) and
the partial product against wo's matching row slice; the host sums the 4
partials per batch (row-parallel wo => partial-sum gather). No collectives.

On-device pipeline per core (bf16 compute, fp32 PSUM accumulation):
  1. q/k/v projections from pre-transposed x, RoPE fused into PSUM-evict
  2. flash-style causal attention in score-transposed layout [keys, queries];
     exp on ScalarE, diagonal blocks trimmed to their causally-live width,
     softmax denominators as a bf16 pair-tree on VectorE + one f32r
     ones-matmul partition-reduce, 1/x as exp(-ln x) on ScalarE,
     partition-broadcast via rank-1 matmul
  3. output projection with back-to-back PSUM accumulation chains, fp32 out
"""
import numpy as np
import ml_dtypes
import orjson
import concourse.bass as bass
import concourse.mybir as mybir
import concourse.tile as tile
from concourse.masks import make_identity
from concourse.bass_utils import run_bass_kernel_spmd

# ---------------------------------------------------------------------------
# Walrus in this image accepts only one sem-wait per instruction; the Tile
# framework's final drain carries several. Split excess waits onto preceding
# NoOps on the same engine (in-order execution preserves the AND semantics).
import orjson

_MARK = "_bir_wait_split_patched"


def split_waits(bir: bytes, maxw: int = 1) -> bytes:
    m = orjson.loads(bir)
    n_split = 0

    def fix_instructions(insts: list) -> list:
        nonlocal n_split
        out = []
        for ins in insts:
            si = ins.get("sync_info")
            waits = (si or {}).get("on_wait") or []
            if len(waits) > maxw:
                n_split += 1
                head, rest = waits[: len(waits) - maxw], waits[len(waits) - maxw :]
                for k in range(0, len(head), maxw):
                    out.append(
                        {
                            "debug": ins.get("debug", 0),
                            "engine": ins["engine"],
                            "ins": [],
                            "name": f"{ins['name']}-wsplit{k}",
                            "opcode": "NoOp",
                            "outs": [],
                            "sync_info": {
                                "on_update": [],
                                "on_wait": head[k : k + maxw],
                            },
                        }
                    )
                si["on_wait"] = rest
            out.append(ins)
        return out

    def walk(o):
        if isinstance(o, dict):
            if isinstance(o.get("instructions"), list):
                o["instructions"] = fix_instructions(o["instructions"])
            for v in o.values():
                walk(v)
        elif isinstance(o, list):
            for v in o:
                walk(v)

    walk(m)
    return orjson.dumps(m)


def patch_nc(nc, maxw: int = 1):
    if getattr(nc, _MARK, False):
        return nc
    orig = nc.to_json_bytes

    def wrapped(*a, **kw):
        return split_waits(orig(*a, **kw), maxw=maxw)

    nc.to_json_bytes = wrapped
    setattr(nc, _MARK, True)
    return nc


# ---------------------------------------------------------------------------
import numpy as np
import ml_dtypes
import concourse.bass as bass
import concourse.mybir as mybir
import concourse.tile as tile
from concourse.masks import make_identity

F32 = mybir.dt.float32
BF16 = mybir.dt.bfloat16
AF = mybir.ActivationFunctionType

P = 128
T = 2048
D = 2048
NT = D // P
HQ = 4
HD = 128
MB = 512
NMB = T // MB
QC = 512
NQC = T // QC
KB = 128
SCALE = float(1.0 / np.sqrt(HD))
HALF = HD // 2


def _rope2(nc, pool, dst, sl, p, cos2, sin2, width):
    """dst[:, sl] = bf16(p*cos2 + swap(p)*sin2); p is fp32 PSUM."""
    sw = pool.tile([P, width], F32, tag="rope_sw", name="sw")
    nc.vector.tensor_copy(sw[0:HALF, :], p[HALF:P, :])
    nc.vector.tensor_copy(sw[HALF:P, :], p[0:HALF, :])
    t2 = pool.tile([P, width], F32, tag="rope_t2", name="t2")
    nc.vector.tensor_mul(t2[:], sw[:], sin2[:, sl])
    t3 = pool.tile([P, width], F32, tag="rope_t3", name="t3")
    nc.vector.tensor_mul(t3[:], p[:], cos2[:, sl])
    nc.vector.tensor_add(dst[:, sl], t3[:], t2[:])


def _phase1(nc, tc, tensors, qT_s, kT_s, v_s, mask_s, ones_c, ones_r, ident):
    (xT_t, wq, wk_t, wv_t, maskp, ones_col_d, ones_row_d, cos2d, sin2d) = tensors
    with (
        tc.tile_pool(name="ph1_w", bufs=1) as w1,
        tc.tile_pool(name="ph1_xt", bufs=3) as xtp,
        tc.tile_pool(name="ph1_ps", bufs=1, space="PSUM") as ps1,
        tc.tile_pool(name="ph1_ptr", bufs=2, space="PSUM") as psp,
        tc.tile_pool(name="ph1_tmp", bufs=3) as rtp,
    ):
        xt0 = xtp.tile([P, NT, MB], BF16, tag="xt", name="xt0")
        nc.sync.dma_start(out=xt0[:, 0 : NT // 2, :], in_=xT_t[:, 0, 0 : NT // 2, :])
        nc.sync.dma_start(out=xt0[:, NT // 2 :, :], in_=xT_t[:, 0, NT // 2 :, :])
        wk_s = w1.tile([P, NT, HD], BF16, name="wk_s")
        nc.sync.dma_start(out=wk_s[:], in_=wk_t)
        wv_s = w1.tile([P, NT, HD], BF16, name="wv_s")
        nc.sync.dma_start(out=wv_s[:], in_=wv_t)
        cos_s = w1.tile([P, T], F32, name="cos_s")
        nc.sync.dma_start(out=cos_s[:], in_=cos2d[:])
        sin_s = w1.tile([P, T], F32, name="sin_s")
        nc.sync.dma_start(out=sin_s[:], in_=sin2d[:])
        wq_s = w1.tile([P, NT, HQ * HD], BF16, name="wq_s")
        wq_t = wq.rearrange("p (t n) -> p t n", t=NT)
        for h in range(HQ):
            nc.sync.dma_start(
                out=wq_s[:, :, h * HD : (h + 1) * HD],
                in_=wq_t[:, :, h * HD : (h + 1) * HD],
            )
        nc.sync.dma_start(out=mask_s[:], in_=maskp[:])
        nc.sync.dma_start(out=ones_c[:].bitcast(mybir.dt.float32r), in_=ones_col_d[:].bitcast(mybir.dt.float32r))
        nc.sync.dma_start(out=ones_r[:], in_=ones_row_d[:])
        make_identity(nc, ident[:])

        for m in range(NMB):
            sl = slice(m * MB, (m + 1) * MB)
            if m == 0:
                xt = xt0
            else:
                xt = xtp.tile([P, NT, MB], BF16, tag="xt", name=f"xt{m}")
                nc.sync.dma_start(out=xt[:, 0 : NT // 2, :], in_=xT_t[:, m, 0 : NT // 2, :])
                nc.sync.dma_start(out=xt[:, NT // 2 :, :], in_=xT_t[:, m, NT // 2 :, :])
            pk = ps1.tile([P, MB], F32, tag="pk", name=f"pk{m}")
            for t in range(NT):
                nc.tensor.matmul(
                    pk[:], wk_s[:, t, :], xt[:, t, :],
                    start=(t == 0), stop=(t == NT - 1),
                )
            _rope2(nc, rtp, kT_s, sl, pk, cos_s, sin_s, MB)
            pv = ps1.tile([P, MB], F32, tag="pv", name=f"pv{m}")
            for t in range(NT):
                nc.tensor.matmul(
                    pv[:], wv_s[:, t, :], xt[:, t, :],
                    start=(t == 0), stop=(t == NT - 1),
                )
            vt_stage = rtp.tile([P, MB], BF16, tag="vts", name=f"vts{m}")
            nc.scalar.copy(vt_stage[:], pv[:])
            for sub in range(MB // P):
                ptr = psp.tile([P, P], BF16, tag="ptr", name=f"ptr{m}_{sub}")
                nc.tensor.transpose(
                    ptr[:], vt_stage[:, sub * P : (sub + 1) * P], ident[:]
                )
                nc.vector.tensor_copy(v_s[:, m * (MB // P) + sub, :], ptr[:])
            for h in range(HQ):
                pq = ps1.tile([P, MB], F32, tag=f"pq{h}", name=f"pq{m}_{h}")
                for t in range(NT):
                    nc.tensor.matmul(
                        pq[:],
                        wq_s[:, t, h * HD : (h + 1) * HD],
                        xt[:, t, :],
                        start=(t == 0),
                        stop=(t == NT - 1),
                    )
                _rope2(nc, rtp, qT_s[h], sl, pq, cos_s, sin_s, MB)


def _attention(nc, tc, qT_s, kT_s, v_s, mask_s, ones_c, ones_r, ctx_s):
    F32R = mybir.dt.float32r

    def r(ap):
        return ap.bitcast(F32R)

    with (
        tc.tile_pool(name="ph2_work", bufs=6) as wk2,
        tc.tile_pool(name="ph2_cp", bufs=2, space="PSUM") as psc,
        tc.tile_pool(name="ph2_sp", bufs=3, space="PSUM") as pss,
        tc.tile_pool(name="ph2_sm", bufs=1, space="PSUM") as psm,
    ):
        for h in range(HQ):
            for c in range(NQC):
                nblk = 4 * (c + 1)
                cp = psc.tile([P, QC], F32, tag="cp", name=f"cp{h}_{c}")
                sumacc = wk2.tile([P, QC], F32, tag="sumacc", name=f"sa{h}_{c}")
                prev = None
                for j in range(nblk):
                    d = j - 4 * c  # >= 0 on diagonal blocks
                    q0 = 128 * d if d >= 0 else 0
                    n = QC - q0
                    qsl = slice(c * QC + q0, (c + 1) * QC)
                    sp = pss.tile([P, QC], F32, tag="sp", name=f"sp{h}_{c}_{j}")
                    nc.tensor.matmul(
                        sp[:, q0:QC],
                        kT_s[:, j * KB : (j + 1) * KB],
                        qT_s[h][:, qsl],
                        start=True, stop=True,
                    )
                    pT = wk2.tile([P, QC], BF16, tag="pT", name=f"pT{h}_{c}_{j}")
                    nc.scalar.activation(pT[:, q0:QC], sp[:, q0:QC], AF.Exp, scale=SCALE)
                    if d >= 0:
                        nc.vector.tensor_mul(
                            pT[:, q0:QC], pT[:, q0:QC], mask_s[:, 3 * KB : 3 * KB + n]
                        )
                    nc.tensor.matmul(
                        cp[:, q0:QC], v_s[:, j, :], pT[:, q0:QC],
                        start=(j == 0), stop=(j == nblk - 1),
                    )
                    # softmax denominators on DVE: pair off-diagonal blocks in
                    # bf16 (2x mode), accumulate pairs + diagonals in fp32
                    if d < 0 and prev is None:
                        prev = pT
                    elif d < 0:
                        pair = wk2.tile([P, QC], BF16, tag="pair", name=f"pp{h}_{c}_{j}")
                        nc.vector.tensor_add(pair[:], prev[:], pT[:])
                        prev = None
                        if j == 1:
                            nc.vector.tensor_copy(r(sumacc[:]), pair[:])
                        else:
                            nc.vector.tensor_add(r(sumacc[:]), sumacc[:], pair[:])
                    else:  # diagonal: accumulate live slice directly in fp32
                        if j == 0:
                            nc.vector.tensor_copy(r(sumacc[:]), pT[:])
                        else:
                            nc.vector.tensor_add(
                                r(sumacc[:, q0:QC]), sumacc[:, q0:QC], pT[:, q0:QC]
                            )
                # partition-reduce on PE (f32r), then 1/x = exp(-ln x) on ACT
                sm = psm.tile([1, QC], F32, tag="sm", name=f"sm{h}_{c}")
                nc.tensor.matmul(sm[:], r(ones_c[:]), r(sumacc[:]), start=True, stop=True)
                lns = wk2.tile([1, QC], F32, tag="lns", name=f"ln{h}_{c}")
                nc.scalar.activation(lns[:], sm[:], AF.Ln)
                rrow = wk2.tile([1, QC], BF16, tag="rrow", name=f"rr{h}_{c}")
                nc.scalar.activation(rrow[:], lns[:], AF.Exp, scale=-1.0)
                prb = psm.tile([P, QC], F32, tag="prb", name=f"prb{h}_{c}")
                nc.tensor.matmul(prb[:], ones_r[:], rrow[:], start=True, stop=True)
                rbc = wk2.tile([P, QC], F32, tag="rbc", name=f"rbc{h}_{c}")
                nc.scalar.copy(rbc[:], prb[:])
                nc.vector.tensor_mul(
                    ctx_s[h][:, c * QC : (c + 1) * QC], cp[:], rbc[:]
                )


def _oproj(nc, tc, ctx_s, wo_s, out):
    with (
        tc.tile_pool(name="ph3_ps", bufs=1, space="PSUM") as ps3,
        tc.tile_pool(name="ph3_out", bufs=6) as outp,
    ):
        for u in range(T // P):
            usl = slice(u * P, (u + 1) * P)
            po = [
                ps3.tile([P, 512], F32, tag=f"po{n}", name=f"po{u}_{n}")
                for n in range(4)
            ]
            for n in range(4):
                for h in range(HQ):
                    nc.tensor.matmul(
                        po[n][:],
                        ctx_s[h][:, usl],
                        wo_s[:, h, n * 512 : (n + 1) * 512],
                        start=(h == 0),
                        stop=(h == HQ - 1),
                    )
            for n in range(4):
                so = outp.tile([P, 512], BF16, tag="so", name=f"so{u}_{n}")
                if n % 2 == 0:
                    nc.vector.tensor_copy(so[:], po[n][:])
                else:
                    nc.scalar.copy(so[:], po[n][:])
                nc.sync.dma_start(out=out[usl, n * 512 : (n + 1) * 512], in_=so[:])


def build():
    nc = bass.Bass()
    xT = nc.declare_dram_parameter("xT", [P, (T // MB) * NT * MB], BF16, isOutput=False)
    wq = nc.declare_dram_parameter("wq", [P, NT * HQ * HD], BF16, isOutput=False)
    wk = nc.declare_dram_parameter("wk", [P, NT * HD], BF16, isOutput=False)
    wv = nc.declare_dram_parameter("wv", [P, NT * HD], BF16, isOutput=False)
    wo = nc.declare_dram_parameter("wo", [P, HQ * D], BF16, isOutput=False)
    cos2d = nc.declare_dram_parameter("cos2", [P, T], F32, isOutput=False)
    sin2d = nc.declare_dram_parameter("sin2", [P, T], F32, isOutput=False)
    maskp = nc.declare_dram_parameter("maskp", [P, 3 * KB + QC], BF16, isOutput=False)
    ones_col_d = nc.declare_dram_parameter("ones_col", [P, 1], F32, isOutput=False)
    ones_row_d = nc.declare_dram_parameter("ones_row", [1, P], BF16, isOutput=False)
    out = nc.declare_dram_parameter("out", [T, D], BF16, isOutput=True)

    xT_t = xT.rearrange("p (m t n) -> p m t n", t=NT, n=MB)
    wk_t = wk.rearrange("p (t n) -> p t n", t=NT)
    wv_t = wv.rearrange("p (t n) -> p t n", t=NT)
    wo_t = wo.rearrange("p (h n) -> p h n", h=HQ)

    with tile.TileContext(nc) as tc, nc.allow_low_precision(reason="bf16 compute"):
        with tc.tile_pool(name="resident", bufs=1) as big:
            qT_s = [big.tile([P, T], BF16, tag=f"qT{h}", name=f"qT{h}") for h in range(HQ)]
            kT_s = big.tile([P, T], BF16, tag="kT", name="kT")
            v_s = big.tile([P, T // P, HD], BF16, tag="v", name="v")
            mask_s = big.tile([P, 3 * KB + QC], BF16, tag="mask", name="mask")
            ones_c = big.tile([P, 1], F32, tag="ones_c", name="ones_c")
            ones_r = big.tile([1, P], BF16, tag="ones_r", name="ones_r")
            ident = big.tile([P, P], BF16, tag="ident", name="ident")

            tensors = (xT_t, wq, wk_t, wv_t, maskp, ones_col_d, ones_row_d, cos2d, sin2d)
            _phase1(nc, tc, tensors, qT_s, kT_s, v_s, mask_s, ones_c, ones_r, ident)

            with tc.tile_pool(name="ph2_sb", bufs=1) as sb2:
                ctx_s = [
                    sb2.tile([P, T], BF16, tag=f"ctx{h}", name=f"ctx{h}")
                    for h in range(HQ)
                ]
                wo_s = sb2.tile([P, HQ, D], BF16, tag="wo", name="wo_s")
                nc.sync.dma_start(out=wo_s[:], in_=wo_t)

                _attention(nc, tc, qT_s, kT_s, v_s, mask_s, ones_c, ones_r, ctx_s)
                _oproj(nc, tc, ctx_s, wo_s, out)

    patch_nc(nc, maxw=1)
    return nc


def rope_tables():
    inv_freq = 1.0 / (10000.0 ** (np.arange(0, HD, 2, dtype=np.float64) / HD))
    t = np.arange(T, dtype=np.float64)
    freqs = np.outer(t, inv_freq)
    c = np.cos(freqs).T.astype(np.float32)
    s = np.sin(freqs).T.astype(np.float32)
    cos2 = np.concatenate([c, c], 0)
    sin2 = np.concatenate([-s, s], 0)
    return np.ascontiguousarray(cos2), np.ascontiguousarray(sin2)


def mask_pad():
    k = np.arange(P)[:, None]
    p = np.arange(3 * KB + QC)[None, :]
    return (p >= k + 3 * KB).astype(ml_dtypes.bfloat16)


def _sbufify(w):
    """[NT*P, N] -> [P, NT*N]: row t*128+p lands at partition p, block t."""
    n = w.shape[1]
    return np.ascontiguousarray(
        w.reshape(NT, P, n).transpose(1, 0, 2).reshape(P, NT * n)
    )


def make_in_maps(x, wq, wk, wv, wo):
    bf = ml_dtypes.bfloat16
    cos2, sin2 = rope_tables()
    maskp = mask_pad()
    ones_col = np.ones((P, 1), np.float32)
    ones_row = np.ones((1, P), bf)
    # x[b]: [T, D] -> [P, NMB*NT*MB]: partition p, chunk m, d-block t holds
    # x[m*MB : (m+1)*MB, t*P+p]  (one contiguous line per chunk per partition)
    xps = []
    for b in range(2):
        xr = x[b].astype(bf).reshape(NMB, MB, NT, P).transpose(3, 0, 2, 1)
        xps.append(np.ascontiguousarray(xr.reshape(P, NMB * NT * MB)))
    wqb, wkb, wvb, wob = (a.astype(bf) for a in (wq, wk, wv, wo))
    in_maps = []
    for c in range(8):
        b, g = divmod(c, 4)
        wog = wob[512 * g : 512 * (g + 1)]  # [512, D]
        wopre = np.ascontiguousarray(
            wog.reshape(HQ, P, D).transpose(1, 0, 2).reshape(P, HQ * D)
        )
        in_maps.append(
            {
                "xT": xps[b],
                "wq": _sbufify(wqb[:, 512 * g : 512 * (g + 1)]),
                "wk": _sbufify(wkb[:, 128 * g : 128 * (g + 1)]),
                "wv": _sbufify(wvb[:, 128 * g : 128 * (g + 1)]),
                "wo": wopre,
                "cos2": cos2,
                "sin2": sin2,
                "maskp": maskp,
                "ones_col": ones_col,
                "ones_row": ones_row,
            }
        )
    return in_maps


def combine_outputs(results):
    out = np.zeros((2, T, D), np.float32)
    for c in range(8):
        out[c // 4] += results[c]["out"].astype(np.float32)
    return out


_NC_CACHE = []


def kernel(x, wq, wk, wv, wo):
    x = np.asarray(x, dtype=np.float32)
    wq = np.asarray(wq, dtype=np.float32)
    wk = np.asarray(wk, dtype=np.float32)
    wv = np.asarray(wv, dtype=np.float32)
    wo = np.asarray(wo, dtype=np.float32)
    if not _NC_CACHE:
        _NC_CACHE.append(build())
    nc = _NC_CACHE[0]
    in_maps = make_in_maps(x, wq, wk, wv, wo)
    res = run_bass_kernel_spmd(nc, in_maps, core_ids=list(range(8)))
    return combine_outputs(res.results)

